# revision 13
# baseline (speedup 1.0000x reference)
"""Trainium2 Bass kernel for MergedColumnParallelLinearWithLoRA.

Computes  out = x @ W.T + concat(lora1(x), lora2(x))  where
lora_i(x)[t] = B_i[l_t] @ (A_i[l_t] @ x[t]) + bias_i[l_t],  l_t = indices[t].

Sharding: ROW-parallel (token-sharded) across 8 NeuronCores. Core c owns
tokens [c*1024, (c+1)*1024); x and indices are sharded along tokens, W /
lora weights are used in full by every core (streamed from HBM). This makes
the LoRA shrink naturally local (no replicated work, no collectives).

Precision strategy (rel-l2 gate is 2e-2):
  - Base GEMM in bf16 (1 cycle/row PE rate, half the HBM traffic of fp32;
    contributes ~2.3e-3 rel error).
  - LoRA shrink + expand in fp8 e4m3 with DoubleRow perf mode (2 k-tiles
    contracted per instruction => ~2x PE throughput on those matmuls). The
    LoRA term is only ~8% of the output magnitude, so fp8's ~3% error there
    adds only ~3e-3 overall. A is pre-scaled by 1/8 and B by 8 on the host
    so both fp8 operands sit in e4m3's normal range.
  - Output written bf16, upconverted to fp32 on the host.

Per-core device program:
  - x^T resident in SBUF twice: bf16 (base) + fp8 (shrink).
  - Chunk 0 = LoRA shrink (fp8 DoubleRow): s1|s2 per token tile -> masked
    dispatch (s_masked = s * (lora_id_col == idx)), PE-transposed into
    resident s^T (stored fp8).
  - Base chunks 1..22 processed in pairs; within a pair the 16 k-matmuls of
    both chunks are interleaved so each x^T stationary is loaded once
    (duplicate InstLdweights removed by _dedupe_ldweights; each Ldweights
    costs ~32 unoverlapped PE cycles on HW).
  - LoRA expand: one fp8 DoubleRow matmul per (chunk, tile) accumulating
    into the base PSUM bank (contracts all 256 s-columns of the slice).
  - Per-token bias rows via indirect-DMA gather (bf16) + DVE add; paired
    chunks share one wide gather and one wide output store per tile.
"""

import numpy as np

import concourse.bass as bass  # noqa: F401
import concourse.mybir as mybir
import concourse.tile as tile
from concourse import bacc
from concourse.masks import make_identity

T, D, O, L, R = 8192, 2048, 5632, 16, 16
NCORES = 8
TL = T // NCORES  # 1024 tokens per core
P = 128
KT = D // P  # 16 k-tiles
MTL = TL // P  # 8 local token tiles
SH = 2 * L * R  # 512 shrink columns (s1 | s2)
NF = 2 * O  # 11264 full output columns
NCH = NF // 512  # 22 base chunks
ASCALE = 8.0  # host pre-scale: A*8, B*8; the transpose matmul divides by 64
F32 = mybir.dt.float32
BF16 = mybir.dt.bfloat16
FP8 = mybir.dt.float8e4
I32 = mybir.dt.int32
DR = mybir.MatmulPerfMode.DoubleRow


def _dedupe_ldweights(nc):
    """Remove InstLdweights that reload the exact stationary AP already in
    the PE array (the paired-chunk loops issue consecutive matmuls sharing
    one stationary). Each Ldweights costs ~32 unoverlapped PE cycles on HW
    (free in the cost model). Safe: a deleted load is byte-identical to the
    one still in the array, and dependency edges are remapped to the kept
    load (where the physical SBUF read actually happens)."""
    n_del = 0
    for blk in nc.main_func.blocks:
        insts = blk.instructions
        last_ld = None
        last_sig = None
        deleted = {}
        keep = []
        for inst in insts:
            eng = str(getattr(inst, "engine", ""))
            if "PE" not in eng:
                keep.append(inst)
                continue
            tn = type(inst).__name__
            if tn == "InstLdweights":
                si = inst.sync_info
                has_sync = si is not None and (
                    len(si.on_wait) > 0 or len(si.on_update) > 0
                )
                sig = str(inst.ins[0])
                if last_sig is not None and sig == last_sig and not has_sync:
                    deleted[inst.name] = last_ld.name
                    n_del += 1
                    continue
                last_ld = inst
                last_sig = sig
                keep.append(inst)
            elif (
                tn == "InstMatmult"
                and inst.ldweights is False
                and not inst.is_transpose
            ):
                keep.append(inst)
            else:
                last_ld = None
                last_sig = None
                keep.append(inst)
        if deleted:
            blk.instructions = keep
            for b2 in nc.main_func.blocks:
                for inst in b2.instructions:
                    inst.remap_dependency_names(deleted)
    return n_del


def build_nc(reps=1, mode="full", bias_via="dma"):
    """mode: 'full' | 'base' (no LoRA shrink/expand)."""
    assert bias_via == "dma"
    nc = bacc.Bacc("TRN2", target_bir_lowering=False, debug=False)

    xt = nc.dram_tensor("xt", [MTL, P, KT, P], BF16, kind="ExternalInput")
    x8 = nc.dram_tensor("x8", [MTL, P, KT, P], FP8, kind="ExternalInput")
    wt = nc.dram_tensor("wt", [NCH, P, KT, 512], BF16, kind="ExternalInput")
    w8 = nc.dram_tensor("w8", [P, KT, 512], FP8, kind="ExternalInput")
    b1 = nc.dram_tensor("b1", [2 * P, O], FP8, kind="ExternalInput")
    b2 = nc.dram_tensor("b2", [2 * P, O], FP8, kind="ExternalInput")
    c1 = nc.dram_tensor("c1", [L, O], BF16, kind="ExternalInput")
    c2 = nc.dram_tensor("c2", [L, O], BF16, kind="ExternalInput")
    idx = nc.dram_tensor("idx", [P, MTL], I32, kind="ExternalInput")
    out = nc.dram_tensor("out", [TL, NF], BF16, kind="ExternalOutput")

    bdram = (b1, b2)
    cdram = (c1, c2)

    with tile.TileContext(nc) as tc:
        with (
            tc.tile_pool(name="const", bufs=1) as const,
            tc.tile_pool(name="wpool", bufs=4) as wpool,
            tc.tile_pool(name="bpool", bufs=3) as bpool,
            tc.tile_pool(name="spool", bufs=3) as spool,
            tc.tile_pool(name="opool", bufs=4) as opool,
            tc.tile_pool(name="gpool", bufs=4) as gpool,
            tc.tile_pool(name="ps_b", bufs=8, space="PSUM") as ps_b,
        ):
            # ---------------- resident constants ----------------
            # The fp8 shrink operands are startup-critical (shrink runs
            # first and is fast); k-slice them so tile-0 matmuls start
            # within ~2us. idx gates the shrink mask (which frees PSUM
            # banks), so it goes first on SP.
            t_idx = const.tile([P, MTL], I32, tag="idxi", name="t_idx")
            nc.sync.dma_start(t_idx[:], idx[:])
            t_idxf = const.tile([P, MTL], F32, tag="idxf", name="t_idxf")
            nc.vector.tensor_copy(t_idxf[:], t_idx[:])

            t_x8 = const.tile([P, MTL, KT, P], FP8, tag="x8", name="t_x8")
            t_w8 = const.tile([P, KT, 512], FP8, tag="w8", name="t_w8")
            for q in range(4):
                sl = slice(4 * q, 4 * q + 4)
                nc.sync.dma_start(t_x8[:, 0, sl], x8[0, :, sl])
                nc.sync.dma_start(t_w8[:, sl], w8[:, sl])
            for mtl in range(1, MTL):
                nc.sync.dma_start(t_x8[:, mtl], x8[mtl])

            # bf16 x^T for the base GEMM rides the Activation queue in
            # parallel (first needed ~10us in, when chunk 1 starts).
            t_xr = const.tile([P, MTL, KT, P], BF16, tag="xr", name="t_xr")
            for mtl in range(MTL):
                nc.scalar.dma_start(t_xr[:, mtl], xt[mtl])

            wtiles = {}

            def _prefetch_w(ch):
                t = wpool.tile([P, KT, 512], BF16, tag="w", name=f"t_w{ch}")
                nc.sync.dma_start(t[:], wt[ch - 1])
                wtiles[ch] = t

            _prefetch_w(1)
            _prefetch_w(2)
            _prefetch_w(3)

            t_identf = const.tile([P, P], F32, tag="identf", name="t_identf")
            make_identity(nc, t_identf[:])
            # identity/64: the shrink computes 8s (A pre-scaled by 8); the
            # transpose matmul rescales so t_st holds s/8, matching B*8.
            t_i64 = const.tile([P, P], BF16, tag="i64", name="t_i64")
            nc.vector.tensor_scalar(
                t_i64[:], t_identf[:], 1.0 / 64, None, op0=mybir.AluOpType.mult
            )

            # lora-id per shrink column: col j (within s1 or s2) -> j // R
            t_lidi = const.tile([P, 2, L, R], I32, tag="lidi", name="t_lidi")
            nc.gpsimd.iota(
                t_lidi[:], pattern=[[0, 2], [1, L], [0, R]], base=0, channel_multiplier=0
            )
            t_lid = const.tile([P, SH], F32, tag="lid", name="t_lid")
            nc.vector.tensor_copy(t_lid[:], t_lidi[:].rearrange("p a l r -> p (a l r)"))

            # resident transposed masked-shrink (fp8 for DoubleRow expand)
            t_st = const.tile([P, MTL, 4 * P], FP8, tag="st", name="t_st")

            for _rep in range(reps):
                # ---------------- chunk 0: LoRA shrink ----------------
                if mode == "full":
                    pend = []

                    def _transpose_sa(mtl, t_sa):
                        p_t = ps_b.tile([P, 4 * P], F32, tag="b", name="p_t")
                        for j in range(4):
                            nc.tensor.matmul(
                                p_t[:, j * P : (j + 1) * P],
                                t_sa[:, j * P : (j + 1) * P],
                                t_i64[:],
                                start=True,
                                stop=True,
                                skip_group_check=True,
                            )
                        nc.vector.tensor_copy(t_st[:, mtl, :], p_t[:])

                    for mtl in range(MTL):
                        p_s = ps_b.tile([P, SH], F32, tag="b", name="p_s")
                        for q in range(KT // 2):
                            nc.tensor.matmul(
                                p_s[:],
                                t_x8[:, mtl, 2 * q : 2 * q + 2, :],
                                t_w8[:, 2 * q : 2 * q + 2, :],
                                start=(q == 0),
                                stop=(q == KT // 2 - 1),
                                perf_mode=DR,
                                skip_group_check=True,
                            )
                        idx_ap = t_idxf[:, mtl : mtl + 1]
                        t_sa = spool.tile([P, SH], BF16, tag="sa", name="t_sa")
                        nc.vector.scalar_tensor_tensor(
                            t_sa[:],
                            t_lid[:],
                            idx_ap,
                            p_s[:],
                            op0=mybir.AluOpType.is_equal,
                            op1=mybir.AluOpType.mult,
                        )
                        pend.append((mtl, t_sa))
                        if len(pend) >= 2:
                            _transpose_sa(*pend.pop(0))
                    while pend:
                        _transpose_sa(*pend.pop(0))

                # ------------- chunks 1..22 as pairs: base + expand -------------
                do_exp = mode == "full"
                for pr in range(NCH // 2):
                    pair = []
                    for ch in (2 * pr + 1, 2 * pr + 2):
                        s, ci = divmod(ch - 1, NCH // 2)
                        if ch in wtiles:
                            t_wc = wtiles.pop(ch)
                        else:
                            t_wc = wpool.tile(
                                [P, KT, 512], BF16, tag="w", name="t_wc"
                            )
                            nc.sync.dma_start(t_wc[:], wt[ch - 1])
                        nxt = ch + 3
                        if nxt <= NCH and nxt not in wtiles:
                            _prefetch_w(nxt)
                        t_b = None
                        if do_exp:
                            t_b = bpool.tile([P, 2, 512], FP8, tag="bb", name="t_b")
                            nc.sync.dma_start(
                                t_b[:],
                                bdram[s][
                                    0 : 2 * P, ci * 512 : (ci + 1) * 512
                                ].rearrange("(c p) o -> p c o", p=P),
                            )
                        pair.append((ch, s, ci, t_wc, t_b))
                    sa, sb = pair[0][1], pair[1][1]
                    straddle = sa != sb
                    for mtl in range(MTL):
                        t_out = opool.tile([P, 2, 512], BF16, tag="o", name="t_out")
                        t_bg = None
                        if do_exp:
                            t_bg = gpool.tile([P, 2, 512], BF16, tag="g", name="t_bg")
                            idx_off = bass.IndirectOffsetOnAxis(
                                ap=t_idx[:, mtl : mtl + 1], axis=0
                            )
                            if straddle:
                                for j, (_, s, ci, _, _) in enumerate(pair):
                                    nc.gpsimd.indirect_dma_start(
                                        out=t_bg[:, j, :],
                                        out_offset=None,
                                        in_=cdram[s][:],
                                        in_offset=idx_off,
                                        element_offset=ci * 512,
                                    )
                            else:
                                nc.gpsimd.indirect_dma_start(
                                    out=t_bg[:].rearrange("p a b -> p (a b)"),
                                    out_offset=None,
                                    in_=cdram[sa][:],
                                    in_offset=idx_off,
                                    element_offset=pair[0][2] * 512,
                                )
                        # kk-interleaved across the pair: both chunks' matmuls
                        # share one x^T stationary per kk (deduped Ldweights)
                        p_ab = [
                            ps_b.tile([P, 512], F32, tag="b", name=f"p_{j}")
                            for j in range(2)
                        ]
                        for kk in range(KT):
                            for j in range(2):
                                nc.tensor.matmul(
                                    p_ab[j][:],
                                    t_xr[:, mtl, kk, :],
                                    pair[j][3][:, kk, :],
                                    start=(kk == 0),
                                    stop=(not do_exp and kk == KT - 1),
                                    skip_group_check=True,
                                )
                        if do_exp:
                            # one fp8 DoubleRow matmul contracts the full 256
                            # s-columns of the slice; paired chunks with the
                            # same slice share the stationary (deduped).
                            for j in range(2):
                                s = pair[j][1]
                                nc.tensor.matmul(
                                    p_ab[j][:],
                                    t_st[
                                        :, mtl, 2 * s * P : (2 * s + 2) * P
                                    ].rearrange("p (h t) -> p h t", h=2),
                                    pair[j][4][:],
                                    start=False,
                                    stop=True,
                                    perf_mode=DR,
                                    skip_group_check=True,
                                )
                        for j, (ch, s, ci, t_wc, t_b) in enumerate(pair):
                            if do_exp:
                                nc.vector.tensor_tensor(
                                    t_out[:, j, :],
                                    p_ab[j][:],
                                    t_bg[:, j, :],
                                    op=mybir.AluOpType.add,
                                )
                            elif (ch + mtl) % 2 == 0:
                                nc.vector.tensor_copy(t_out[:, j, :], p_ab[j][:])
                            else:
                                nc.scalar.copy(t_out[:, j, :], p_ab[j][:])
                        nc.scalar.dma_start(
                            out[
                                mtl * P : (mtl + 1) * P,
                                2 * pr * 512 : (2 * pr + 2) * 512,
                            ],
                            t_out[:].rearrange("p a b -> p (a b)"),
                        )

    n_del = _dedupe_ldweights(nc)
    print(f"deduped {n_del} InstLdweights")
    nc.compile()
    return nc


# ---------------------------------------------------------------------------
# host-side sharding / unsharding
# ---------------------------------------------------------------------------


def _bf16(a):
    import ml_dtypes

    return np.asarray(a, np.float32).astype(ml_dtypes.bfloat16)


def _fp8(a):
    import ml_dtypes

    return np.asarray(a, np.float32).astype(ml_dtypes.float8_e4m3)


def shard_inputs(x, W, lora_a1, lora_a2, lora_b1, lora_b2, bias1, bias2, indices):
    indices = np.asarray(indices, np.int32)

    # base weight chunks: wt[ch, p, kk, j] = W[ch*512 + j, kk*128 + p]
    Wf = np.asarray(W, np.float32)
    wt = _bf16(
        np.ascontiguousarray(Wf.T.reshape(KT, P, NCH, 512).transpose(2, 1, 0, 3))
    )

    # fp8 shrink weights (A1|A2 flattened, pre-scaled by 1/ASCALE)
    a1f = np.asarray(lora_a1, np.float32).reshape(L * R, D)
    a2f = np.asarray(lora_a2, np.float32).reshape(L * R, D)
    aaug = np.concatenate([a1f, a2f], axis=0) * ASCALE  # [512, D]
    w8 = _fp8(np.ascontiguousarray(aaug.T.reshape(KT, P, 512).transpose(1, 0, 2)))

    def bmat(lb):
        bf = np.asarray(lb, np.float32).transpose(0, 2, 1).reshape(L * R, O)
        return _fp8(bf * ASCALE)

    b1m = bmat(lora_b1)
    b2m = bmat(lora_b2)

    # xt[c][mtl, p, kk, m] = x[c*1024 + mtl*128 + m, kk*128 + p]
    xtt = np.asarray(x, np.float32).reshape(NCORES, MTL, P, KT, P).transpose(
        0, 1, 4, 3, 2
    )
    xts = _bf16(xtt)
    x8s = _fp8(xtt)
    idxs = indices.reshape(NCORES, MTL, P).transpose(0, 2, 1)

    c1m = _bf16(bias1)
    c2m = _bf16(bias2)

    in_maps = []
    for c in range(NCORES):
        in_maps.append(
            {
                "xt": np.ascontiguousarray(xts[c]),
                "x8": np.ascontiguousarray(x8s[c]),
                "wt": wt,
                "w8": w8,
                "b1": b1m,
                "b2": b2m,
                "c1": c1m,
                "c2": c2m,
                "idx": np.ascontiguousarray(idxs[c]),
            }
        )
    return in_maps


def unshard_output(results):
    out = np.empty((T, NF), np.float32)
    for c in range(NCORES):
        out[c * TL : (c + 1) * TL, :] = np.asarray(results[c]["out"], np.float32)
    return out


_CACHE = {}


def get_nc():
    if "nc" not in _CACHE:
        _CACHE["nc"] = build_nc()
    return _CACHE["nc"]


def kernel(**inputs):
    from concourse import bass2jax

    nc = get_nc()
    in_maps = shard_inputs(**inputs)
    results = bass2jax.run_bass_via_pjrt(nc, in_maps, n_cores=NCORES)
    return unshard_output(results)


# revision 14
# speedup vs baseline: 1.0292x; 1.0292x over previous
"""Trainium2 Bass kernel for MergedColumnParallelLinearWithLoRA.

Computes  out = x @ W.T + concat(lora1(x), lora2(x))  where
lora_i(x)[t] = B_i[l_t] @ (A_i[l_t] @ x[t]) + bias_i[l_t],  l_t = indices[t].

Sharding: ROW-parallel (token-sharded) across 8 NeuronCores. Core c owns
tokens [c*1024, (c+1)*1024); x and indices are sharded along tokens, W /
lora weights are used in full by every core (streamed from HBM). This makes
the LoRA shrink naturally local (no replicated work, no collectives).

Precision strategy (rel-l2 gate is 2e-2):
  - Base GEMM in bf16 (1 cycle/row PE rate, half the HBM traffic of fp32;
    contributes ~2.3e-3 rel error).
  - LoRA shrink + expand in fp8 e4m3 with DoubleRow perf mode (2 k-tiles
    contracted per instruction => ~2x PE throughput on those matmuls). The
    LoRA term is only ~8% of the output magnitude, so fp8's ~3% error there
    adds only ~3e-3 overall. A is pre-scaled by 1/8 and B by 8 on the host
    so both fp8 operands sit in e4m3's normal range.
  - Output written bf16, upconverted to fp32 on the host.

Per-core device program:
  - x^T resident in SBUF twice: bf16 (base) + fp8 (shrink).
  - Chunk 0 = LoRA shrink (fp8 DoubleRow): s1|s2 per token tile -> masked
    dispatch (s_masked = s * (lora_id_col == idx)), PE-transposed into
    resident s^T (stored fp8).
  - Base chunks 1..22 processed in pairs; within a pair the 16 k-matmuls of
    both chunks are interleaved so each x^T stationary is loaded once
    (duplicate InstLdweights removed by _dedupe_ldweights; each Ldweights
    costs ~32 unoverlapped PE cycles on HW).
  - LoRA expand: one fp8 DoubleRow matmul per (chunk, tile) accumulating
    into the base PSUM bank (contracts all 256 s-columns of the slice).
  - Per-token bias rows via indirect-DMA gather (bf16) + DVE add; paired
    chunks share one wide gather and one wide output store per tile.
"""

import numpy as np

import concourse.bass as bass  # noqa: F401
import concourse.mybir as mybir
import concourse.tile as tile
from concourse import bacc
from concourse.masks import make_identity

T, D, O, L, R = 8192, 2048, 5632, 16, 16
NCORES = 8
TL = T // NCORES  # 1024 tokens per core
P = 128
KT = D // P  # 16 k-tiles
MTL = TL // P  # 8 local token tiles
SH = 2 * L * R  # 512 shrink columns (s1 | s2)
NF = 2 * O  # 11264 full output columns
NCH = NF // 512  # 22 base chunks
ASCALE = 8.0  # host pre-scale: A*8, B*8; the transpose matmul divides by 64
F32 = mybir.dt.float32
BF16 = mybir.dt.bfloat16
FP8 = mybir.dt.float8e4
I32 = mybir.dt.int32
DR = mybir.MatmulPerfMode.DoubleRow


def _dedupe_ldweights(nc):
    """Remove InstLdweights that reload the exact stationary AP already in
    the PE array (the paired-chunk loops issue consecutive matmuls sharing
    one stationary). Each Ldweights costs ~32 unoverlapped PE cycles on HW
    (free in the cost model). Safe: a deleted load is byte-identical to the
    one still in the array, and dependency edges are remapped to the kept
    load (where the physical SBUF read actually happens)."""
    n_del = 0
    for blk in nc.main_func.blocks:
        insts = blk.instructions
        last_ld = None
        last_sig = None
        deleted = {}
        keep = []
        for inst in insts:
            eng = str(getattr(inst, "engine", ""))
            if "PE" not in eng:
                keep.append(inst)
                continue
            tn = type(inst).__name__
            if tn == "InstLdweights":
                si = inst.sync_info
                has_sync = si is not None and (
                    len(si.on_wait) > 0 or len(si.on_update) > 0
                )
                sig = str(inst.ins[0])
                if last_sig is not None and sig == last_sig and not has_sync:
                    deleted[inst.name] = last_ld.name
                    n_del += 1
                    continue
                last_ld = inst
                last_sig = sig
                keep.append(inst)
            elif (
                tn == "InstMatmult"
                and inst.ldweights is False
                and not inst.is_transpose
            ):
                keep.append(inst)
            else:
                last_ld = None
                last_sig = None
                keep.append(inst)
        if deleted:
            blk.instructions = keep
            for b2 in nc.main_func.blocks:
                for inst in b2.instructions:
                    inst.remap_dependency_names(deleted)
    return n_del


def build_nc(reps=1, mode="full", bias_via="dma"):
    """mode: 'full' | 'base' (no LoRA shrink/expand)."""
    assert bias_via == "dma"
    nc = bacc.Bacc("TRN2", target_bir_lowering=False, debug=False)

    xt = nc.dram_tensor("xt", [MTL, P, KT, P], BF16, kind="ExternalInput")
    x8 = nc.dram_tensor("x8", [MTL, P, KT, P], FP8, kind="ExternalInput")
    wt = nc.dram_tensor("wt", [NCH, P, KT, 512], BF16, kind="ExternalInput")
    w8 = nc.dram_tensor("w8", [P, KT, 512], FP8, kind="ExternalInput")
    b1 = nc.dram_tensor("b1", [2 * P, O], FP8, kind="ExternalInput")
    b2 = nc.dram_tensor("b2", [2 * P, O], FP8, kind="ExternalInput")
    c1 = nc.dram_tensor("c1", [L, O], BF16, kind="ExternalInput")
    c2 = nc.dram_tensor("c2", [L, O], BF16, kind="ExternalInput")
    idx = nc.dram_tensor("idx", [P, MTL], I32, kind="ExternalInput")
    out = nc.dram_tensor("out", [TL, NF], BF16, kind="ExternalOutput")

    bdram = (b1, b2)
    cdram = (c1, c2)

    with tile.TileContext(nc) as tc:
        with (
            tc.tile_pool(name="const", bufs=1) as const,
            tc.tile_pool(name="wpool", bufs=4) as wpool,
            tc.tile_pool(name="bpool", bufs=3) as bpool,
            tc.tile_pool(name="spool", bufs=3) as spool,
            tc.tile_pool(name="opool", bufs=4) as opool,
            tc.tile_pool(name="gpool", bufs=4) as gpool,
            tc.tile_pool(name="ps_b", bufs=8, space="PSUM") as ps_b,
        ):
            # ---------------- resident constants ----------------
            # The fp8 shrink operands are startup-critical (shrink runs
            # first and is fast); k-slice them so tile-0 matmuls start
            # within ~2us. idx gates the shrink mask (which frees PSUM
            # banks), so it goes first on SP.
            t_idx = const.tile([P, MTL], I32, tag="idxi", name="t_idx")
            nc.sync.dma_start(t_idx[:], idx[:])
            t_idxf = const.tile([P, MTL], F32, tag="idxf", name="t_idxf")
            nc.vector.tensor_copy(t_idxf[:], t_idx[:])

            t_x8 = const.tile([P, MTL, KT, P], FP8, tag="x8", name="t_x8")
            t_w8 = const.tile([P, KT, 512], FP8, tag="w8", name="t_w8")
            for q in range(4):
                sl = slice(4 * q, 4 * q + 4)
                nc.sync.dma_start(t_x8[:, 0, sl], x8[0, :, sl])
                nc.sync.dma_start(t_w8[:, sl], w8[:, sl])
            for mtl in range(1, MTL):
                nc.sync.dma_start(t_x8[:, mtl], x8[mtl])

            # bf16 x^T for the base GEMM rides the Activation queue in
            # parallel (first needed ~10us in, when chunk 1 starts).
            t_xr = const.tile([P, MTL, KT, P], BF16, tag="xr", name="t_xr")
            for mtl in range(MTL):
                nc.scalar.dma_start(t_xr[:, mtl], xt[mtl])

            wtiles = {}

            def _prefetch_w(ch):
                t = wpool.tile([P, KT, 512], BF16, tag="w", name=f"t_w{ch}")
                nc.sync.dma_start(t[:], wt[ch - 1])
                wtiles[ch] = t

            _prefetch_w(1)
            _prefetch_w(2)
            _prefetch_w(3)

            t_identf = const.tile([P, P], F32, tag="identf", name="t_identf")
            make_identity(nc, t_identf[:])
            # identity/64: the shrink computes 8s (A pre-scaled by 8); the
            # transpose matmul rescales so t_st holds s/8, matching B*8.
            t_i64 = const.tile([P, P], BF16, tag="i64", name="t_i64")
            nc.vector.tensor_scalar(
                t_i64[:], t_identf[:], 1.0 / 64, None, op0=mybir.AluOpType.mult
            )

            # lora-id per shrink column: col j (within s1 or s2) -> j // R
            t_lidi = const.tile([P, 2, L, R], I32, tag="lidi", name="t_lidi")
            nc.gpsimd.iota(
                t_lidi[:], pattern=[[0, 2], [1, L], [0, R]], base=0, channel_multiplier=0
            )
            t_lid = const.tile([P, SH], F32, tag="lid", name="t_lid")
            nc.vector.tensor_copy(t_lid[:], t_lidi[:].rearrange("p a l r -> p (a l r)"))

            # resident transposed masked-shrink (fp8 for DoubleRow expand)
            t_st = const.tile([P, MTL, 4 * P], FP8, tag="st", name="t_st")

            for _rep in range(reps):
                # ---------------- chunk 0: LoRA shrink ----------------
                if mode == "full":
                    pend = []

                    def _transpose_sa(mtl, t_sa):
                        p_t = ps_b.tile([P, 4 * P], F32, tag="b", name="p_t")
                        for j in range(4):
                            nc.tensor.matmul(
                                p_t[:, j * P : (j + 1) * P],
                                t_sa[:, j * P : (j + 1) * P],
                                t_i64[:],
                                start=True,
                                stop=True,
                                skip_group_check=True,
                            )
                        nc.vector.tensor_copy(t_st[:, mtl, :], p_t[:])

                    for mtl in range(MTL):
                        p_s = ps_b.tile([P, SH], F32, tag="b", name="p_s")
                        for q in range(KT // 2):
                            nc.tensor.matmul(
                                p_s[:],
                                t_x8[:, mtl, 2 * q : 2 * q + 2, :],
                                t_w8[:, 2 * q : 2 * q + 2, :],
                                start=(q == 0),
                                stop=(q == KT // 2 - 1),
                                perf_mode=DR,
                                skip_group_check=True,
                            )
                        idx_ap = t_idxf[:, mtl : mtl + 1]
                        t_sa = spool.tile([P, SH], BF16, tag="sa", name="t_sa")
                        nc.vector.scalar_tensor_tensor(
                            t_sa[:],
                            t_lid[:],
                            idx_ap,
                            p_s[:],
                            op0=mybir.AluOpType.is_equal,
                            op1=mybir.AluOpType.mult,
                        )
                        pend.append((mtl, t_sa))
                        if len(pend) >= 2:
                            _transpose_sa(*pend.pop(0))
                    while pend:
                        _transpose_sa(*pend.pop(0))

                # ------------- chunks 1..22 as pairs: base + expand -------------
                do_exp = mode == "full"
                for pr in range(NCH // 2):
                    pair = []
                    for ch in (2 * pr + 1, 2 * pr + 2):
                        s, ci = divmod(ch - 1, NCH // 2)
                        if ch in wtiles:
                            t_wc = wtiles.pop(ch)
                        else:
                            t_wc = wpool.tile(
                                [P, KT, 512], BF16, tag="w", name="t_wc"
                            )
                            nc.sync.dma_start(t_wc[:], wt[ch - 1])
                        nxt = ch + 3
                        if nxt <= NCH and nxt not in wtiles:
                            _prefetch_w(nxt)
                        t_b = None
                        if do_exp:
                            t_b = bpool.tile([P, 2, 512], FP8, tag="bb", name="t_b")
                            nc.sync.dma_start(
                                t_b[:],
                                bdram[s][
                                    0 : 2 * P, ci * 512 : (ci + 1) * 512
                                ].rearrange("(c p) o -> p c o", p=P),
                            )
                        pair.append((ch, s, ci, t_wc, t_b))
                    sa, sb = pair[0][1], pair[1][1]
                    straddle = sa != sb
                    for mtl in range(MTL):
                        t_out = opool.tile([P, 2, 512], BF16, tag="o", name="t_out")
                        t_bg = None
                        if do_exp:
                            t_bg = gpool.tile([P, 2, 512], BF16, tag="g", name="t_bg")
                            idx_off = bass.IndirectOffsetOnAxis(
                                ap=t_idx[:, mtl : mtl + 1], axis=0
                            )
                            if straddle:
                                for j, (_, s, ci, _, _) in enumerate(pair):
                                    nc.gpsimd.indirect_dma_start(
                                        out=t_bg[:, j, :],
                                        out_offset=None,
                                        in_=cdram[s][:],
                                        in_offset=idx_off,
                                        element_offset=ci * 512,
                                    )
                            else:
                                nc.gpsimd.indirect_dma_start(
                                    out=t_bg[:].rearrange("p a b -> p (a b)"),
                                    out_offset=None,
                                    in_=cdram[sa][:],
                                    in_offset=idx_off,
                                    element_offset=pair[0][2] * 512,
                                )
                        # kk-interleaved across the pair: both chunks' matmuls
                        # share one x^T stationary per kk (deduped Ldweights)
                        p_ab = [
                            ps_b.tile([P, 512], F32, tag="b", name=f"p_{j}")
                            for j in range(2)
                        ]
                        for kk in range(KT):
                            for j in range(2):
                                nc.tensor.matmul(
                                    p_ab[j][:],
                                    t_xr[:, mtl, kk, :],
                                    pair[j][3][:, kk, :],
                                    start=(kk == 0),
                                    stop=(not do_exp and kk == KT - 1),
                                    skip_group_check=True,
                                )
                        if do_exp:
                            # one fp8 DoubleRow matmul contracts the full 256
                            # s-columns of the slice; paired chunks with the
                            # same slice share the stationary (deduped).
                            for j in range(2):
                                s = pair[j][1]
                                nc.tensor.matmul(
                                    p_ab[j][:],
                                    t_st[
                                        :, mtl, 2 * s * P : (2 * s + 2) * P
                                    ].rearrange("p (h t) -> p h t", h=2),
                                    pair[j][4][:],
                                    start=False,
                                    stop=True,
                                    perf_mode=DR,
                                    skip_group_check=True,
                                )
                        for j, (ch, s, ci, t_wc, t_b) in enumerate(pair):
                            if do_exp:
                                nc.vector.tensor_tensor(
                                    t_out[:, j, :],
                                    p_ab[j][:],
                                    t_bg[:, j, :],
                                    op=mybir.AluOpType.add,
                                )
                            elif (ch + mtl) % 2 == 0:
                                nc.vector.tensor_copy(t_out[:, j, :], p_ab[j][:])
                            else:
                                nc.scalar.copy(t_out[:, j, :], p_ab[j][:])
                        nc.scalar.dma_start(
                            out[
                                mtl * P : (mtl + 1) * P,
                                2 * pr * 512 : (2 * pr + 2) * 512,
                            ],
                            t_out[:].rearrange("p a b -> p (a b)"),
                        )

    import os

    if os.environ.get("KERNEL_NO_DEDUPE"):
        print("dedupe disabled via KERNEL_NO_DEDUPE")
    else:
        n_del = _dedupe_ldweights(nc)
        print(f"deduped {n_del} InstLdweights")
    nc.compile()
    return nc


# ---------------------------------------------------------------------------
# host-side sharding / unsharding
# ---------------------------------------------------------------------------


def _bf16(a):
    import ml_dtypes

    return np.asarray(a, np.float32).astype(ml_dtypes.bfloat16)


def _fp8(a):
    import ml_dtypes

    return np.asarray(a, np.float32).astype(ml_dtypes.float8_e4m3)


def shard_inputs(x, W, lora_a1, lora_a2, lora_b1, lora_b2, bias1, bias2, indices):
    indices = np.asarray(indices, np.int32)

    # base weight chunks: wt[ch, p, kk, j] = W[ch*512 + j, kk*128 + p]
    Wf = np.asarray(W, np.float32)
    wt = _bf16(
        np.ascontiguousarray(Wf.T.reshape(KT, P, NCH, 512).transpose(2, 1, 0, 3))
    )

    # fp8 shrink weights (A1|A2 flattened, pre-scaled by 1/ASCALE)
    a1f = np.asarray(lora_a1, np.float32).reshape(L * R, D)
    a2f = np.asarray(lora_a2, np.float32).reshape(L * R, D)
    aaug = np.concatenate([a1f, a2f], axis=0) * ASCALE  # [512, D]
    w8 = _fp8(np.ascontiguousarray(aaug.T.reshape(KT, P, 512).transpose(1, 0, 2)))

    def bmat(lb):
        bf = np.asarray(lb, np.float32).transpose(0, 2, 1).reshape(L * R, O)
        return _fp8(bf * ASCALE)

    b1m = bmat(lora_b1)
    b2m = bmat(lora_b2)

    # xt[c][mtl, p, kk, m] = x[c*1024 + mtl*128 + m, kk*128 + p]
    xtt = np.asarray(x, np.float32).reshape(NCORES, MTL, P, KT, P).transpose(
        0, 1, 4, 3, 2
    )
    xts = _bf16(xtt)
    x8s = _fp8(xtt)
    idxs = indices.reshape(NCORES, MTL, P).transpose(0, 2, 1)

    c1m = _bf16(bias1)
    c2m = _bf16(bias2)

    in_maps = []
    for c in range(NCORES):
        in_maps.append(
            {
                "xt": np.ascontiguousarray(xts[c]),
                "x8": np.ascontiguousarray(x8s[c]),
                "wt": wt,
                "w8": w8,
                "b1": b1m,
                "b2": b2m,
                "c1": c1m,
                "c2": c2m,
                "idx": np.ascontiguousarray(idxs[c]),
            }
        )
    return in_maps


def unshard_output(results):
    out = np.empty((T, NF), np.float32)
    for c in range(NCORES):
        out[c * TL : (c + 1) * TL, :] = np.asarray(results[c]["out"], np.float32)
    return out


_CACHE = {}


def get_nc():
    if "nc" not in _CACHE:
        _CACHE["nc"] = build_nc()
    return _CACHE["nc"]


def kernel(**inputs):
    from concourse import bass2jax

    nc = get_nc()
    in_maps = shard_inputs(**inputs)
    results = bass2jax.run_bass_via_pjrt(nc, in_maps, n_cores=NCORES)
    return unshard_output(results)


# revision 15
# speedup vs baseline: 1.1918x; 1.1580x over previous
"""Trainium2 Bass kernel for MergedColumnParallelLinearWithLoRA.

Computes  out = x @ W.T + concat(lora1(x), lora2(x))  where
lora_i(x)[t] = B_i[l_t] @ (A_i[l_t] @ x[t]) + bias_i[l_t],  l_t = indices[t].

Sharding: ROW-parallel (token-sharded) across 8 NeuronCores. Core c owns
tokens [c*1024, (c+1)*1024); x and indices are sharded along tokens, W /
lora weights are used in full by every core (streamed from HBM). This makes
the LoRA shrink naturally local (no replicated work, no collectives).

Precision strategy (rel-l2 gate is 2e-2):
  - Base GEMM in bf16 (1 cycle/row PE rate, half the HBM traffic of fp32;
    contributes ~2.3e-3 rel error).
  - LoRA shrink + expand in fp8 e4m3 with DoubleRow perf mode (2 k-tiles
    contracted per instruction => ~2x PE throughput on those matmuls). The
    LoRA term is only ~8% of the output magnitude, so fp8's ~3% error there
    adds only ~3e-3 overall. A is pre-scaled by 1/8 and B by 8 on the host
    so both fp8 operands sit in e4m3's normal range.
  - Output written bf16, upconverted to fp32 on the host.

Per-core device program:
  - x^T resident in SBUF twice: bf16 (base) + fp8 (shrink).
  - Chunk 0 = LoRA shrink (fp8 DoubleRow): s1|s2 per token tile -> masked
    dispatch (s_masked = s * (lora_id_col == idx)), PE-transposed into
    resident s^T (stored fp8).
  - Base chunks 1..22 processed in pairs; within a pair the 16 k-matmuls of
    both chunks are interleaved so each x^T stationary is loaded once
    (duplicate InstLdweights removed by _dedupe_ldweights; each Ldweights
    costs ~32 unoverlapped PE cycles on HW).
  - LoRA expand: one fp8 DoubleRow matmul per (chunk, tile) accumulating
    into the base PSUM bank (contracts all 256 s-columns of the slice).
  - Per-token bias rows via indirect-DMA gather (bf16) + DVE add; paired
    chunks share one wide gather and one wide output store per tile.
"""

import numpy as np

import concourse.bass as bass  # noqa: F401
import concourse.mybir as mybir
import concourse.tile as tile
from concourse import bacc
from concourse.masks import make_identity

T, D, O, L, R = 8192, 2048, 5632, 16, 16
NCORES = 8
TL = T // NCORES  # 1024 tokens per core
P = 128
KT = D // P  # 16 k-tiles
MTL = TL // P  # 8 local token tiles
SH = 2 * L * R  # 512 shrink columns (s1 | s2)
NF = 2 * O  # 11264 full output columns
NCH = NF // 512  # 22 base chunks
ASCALE = 8.0  # host pre-scale: A*8, B*8; the transpose matmul divides by 64
F32 = mybir.dt.float32
BF16 = mybir.dt.bfloat16
FP8 = mybir.dt.float8e4
I32 = mybir.dt.int32
DR = mybir.MatmulPerfMode.DoubleRow


def _dedupe_ldweights(nc):
    """Remove InstLdweights that reload the exact stationary AP already in
    the PE array (the paired-chunk loops issue consecutive matmuls sharing
    one stationary). Each Ldweights costs ~32 unoverlapped PE cycles on HW
    (free in the cost model). Safe: a deleted load is byte-identical to the
    one still in the array, and dependency edges are remapped to the kept
    load (where the physical SBUF read actually happens)."""
    n_del = 0
    for blk in nc.main_func.blocks:
        insts = blk.instructions
        last_ld = None
        last_sig = None
        deleted = {}
        keep = []
        for inst in insts:
            eng = str(getattr(inst, "engine", ""))
            if "PE" not in eng:
                keep.append(inst)
                continue
            tn = type(inst).__name__
            if tn == "InstLdweights":
                si = inst.sync_info
                has_sync = si is not None and (
                    len(si.on_wait) > 0 or len(si.on_update) > 0
                )
                sig = str(inst.ins[0])
                if last_sig is not None and sig == last_sig and not has_sync:
                    deleted[inst.name] = last_ld.name
                    n_del += 1
                    continue
                last_ld = inst
                last_sig = sig
                keep.append(inst)
            elif (
                tn == "InstMatmult"
                and inst.ldweights is False
                and not inst.is_transpose
            ):
                keep.append(inst)
            else:
                last_ld = None
                last_sig = None
                keep.append(inst)
        if deleted:
            blk.instructions = keep
            for b2 in nc.main_func.blocks:
                for inst in b2.instructions:
                    inst.remap_dependency_names(deleted)
    return n_del


def build_nc(reps=1, mode="full", bias_via="dma"):
    """mode: 'full' | 'base' (no LoRA shrink/expand)."""
    assert bias_via == "dma"
    nc = bacc.Bacc("TRN2", target_bir_lowering=False, debug=False)

    xt = nc.dram_tensor("xt", [MTL, P, KT, P], BF16, kind="ExternalInput")
    x8 = nc.dram_tensor("x8", [MTL, P, KT, P], FP8, kind="ExternalInput")
    wt = nc.dram_tensor("wt", [NCH, P, KT, 512], BF16, kind="ExternalInput")
    w8 = nc.dram_tensor("w8", [P, KT, 512], FP8, kind="ExternalInput")
    b1 = nc.dram_tensor("b1", [2 * P, O], FP8, kind="ExternalInput")
    b2 = nc.dram_tensor("b2", [2 * P, O], FP8, kind="ExternalInput")
    c1 = nc.dram_tensor("c1", [L, O], BF16, kind="ExternalInput")
    c2 = nc.dram_tensor("c2", [L, O], BF16, kind="ExternalInput")
    idx = nc.dram_tensor("idx", [P, MTL], I32, kind="ExternalInput")
    out = nc.dram_tensor("out", [TL, NF], BF16, kind="ExternalOutput")

    bdram = (b1, b2)
    cdram = (c1, c2)

    with tile.TileContext(nc) as tc:
        with (
            tc.tile_pool(name="const", bufs=1) as const,
            tc.tile_pool(name="wpool", bufs=4) as wpool,
            tc.tile_pool(name="bpool", bufs=3) as bpool,
            tc.tile_pool(name="spool", bufs=3) as spool,
            tc.tile_pool(name="opool", bufs=4) as opool,
            tc.tile_pool(name="gpool", bufs=4) as gpool,
            tc.tile_pool(name="ps_b", bufs=8, space="PSUM") as ps_b,
        ):
            # ---------------- resident constants ----------------
            # The fp8 shrink operands are startup-critical (shrink runs
            # first and is fast); k-slice them so tile-0 matmuls start
            # within ~2us. idx gates the shrink mask (which frees PSUM
            # banks), so it goes first on SP.
            t_idx = const.tile([P, MTL], I32, tag="idxi", name="t_idx")
            nc.sync.dma_start(t_idx[:], idx[:])
            t_idxf = const.tile([P, MTL], F32, tag="idxf", name="t_idxf")
            nc.vector.tensor_copy(t_idxf[:], t_idx[:])

            t_x8 = const.tile([P, MTL, KT, P], FP8, tag="x8", name="t_x8")
            t_w8 = const.tile([P, KT, 512], FP8, tag="w8", name="t_w8")
            for q in range(4):
                sl = slice(4 * q, 4 * q + 4)
                nc.sync.dma_start(t_x8[:, 0, sl], x8[0, :, sl])
                nc.sync.dma_start(t_w8[:, sl], w8[:, sl])
            for mtl in range(1, MTL):
                nc.gpsimd.dma_start(t_x8[:, mtl], x8[mtl])

            # bf16 x^T for the base GEMM rides the Activation queue in
            # parallel (first needed ~10us in, when chunk 1 starts).
            t_xr = const.tile([P, MTL, KT, P], BF16, tag="xr", name="t_xr")
            for mtl in range(MTL):
                nc.scalar.dma_start(t_xr[:, mtl], xt[mtl])

            wtiles = {}

            def _prefetch_w(ch):
                t = wpool.tile([P, KT, 512], BF16, tag="w", name=f"t_w{ch}")
                nc.sync.dma_start(t[:], wt[ch - 1])
                wtiles[ch] = t

            _prefetch_w(1)
            _prefetch_w(2)
            _prefetch_w(3)

            t_identf = const.tile([P, P], F32, tag="identf", name="t_identf")
            make_identity(nc, t_identf[:])
            # identity/64: the shrink computes 8s (A pre-scaled by 8); the
            # transpose matmul rescales so t_st holds s/8, matching B*8.
            t_i64 = const.tile([P, P], BF16, tag="i64", name="t_i64")
            nc.vector.tensor_scalar(
                t_i64[:], t_identf[:], 1.0 / 64, None, op0=mybir.AluOpType.mult
            )

            # lora-id per shrink column: col j (within s1 or s2) -> j // R
            t_lidi = const.tile([P, 2, L, R], I32, tag="lidi", name="t_lidi")
            nc.gpsimd.iota(
                t_lidi[:], pattern=[[0, 2], [1, L], [0, R]], base=0, channel_multiplier=0
            )
            t_lid = const.tile([P, SH], F32, tag="lid", name="t_lid")
            nc.vector.tensor_copy(t_lid[:], t_lidi[:].rearrange("p a l r -> p (a l r)"))

            # resident transposed masked-shrink (fp8 for DoubleRow expand)
            t_st = const.tile([P, MTL, 4 * P], FP8, tag="st", name="t_st")

            for _rep in range(reps):
                # ---------------- chunk 0: LoRA shrink ----------------
                if mode == "full":
                    pend = []

                    def _transpose_sa(mtl, t_sa):
                        p_t = ps_b.tile([P, 4 * P], F32, tag="b", name="p_t")
                        for j in range(4):
                            nc.tensor.matmul(
                                p_t[:, j * P : (j + 1) * P],
                                t_sa[:, j * P : (j + 1) * P],
                                t_i64[:],
                                start=True,
                                stop=True,
                                skip_group_check=True,
                            )
                        nc.vector.tensor_copy(t_st[:, mtl, :], p_t[:])

                    for mtl in range(MTL):
                        p_s = ps_b.tile([P, SH], F32, tag="b", name="p_s")
                        for q in range(KT // 2):
                            nc.tensor.matmul(
                                p_s[:],
                                t_x8[:, mtl, 2 * q : 2 * q + 2, :],
                                t_w8[:, 2 * q : 2 * q + 2, :],
                                start=(q == 0),
                                stop=(q == KT // 2 - 1),
                                perf_mode=DR,
                                skip_group_check=True,
                            )
                        idx_ap = t_idxf[:, mtl : mtl + 1]
                        t_sa = spool.tile([P, SH], BF16, tag="sa", name="t_sa")
                        nc.vector.scalar_tensor_tensor(
                            t_sa[:],
                            t_lid[:],
                            idx_ap,
                            p_s[:],
                            op0=mybir.AluOpType.is_equal,
                            op1=mybir.AluOpType.mult,
                        )
                        pend.append((mtl, t_sa))
                        if len(pend) >= 2:
                            _transpose_sa(*pend.pop(0))
                    while pend:
                        _transpose_sa(*pend.pop(0))

                # ------------- chunks 1..22 as pairs: base + expand -------------
                do_exp = mode == "full"
                for pr in range(NCH // 2):
                    pair = []
                    for ch in (2 * pr + 1, 2 * pr + 2):
                        s, ci = divmod(ch - 1, NCH // 2)
                        if ch in wtiles:
                            t_wc = wtiles.pop(ch)
                        else:
                            t_wc = wpool.tile(
                                [P, KT, 512], BF16, tag="w", name="t_wc"
                            )
                            nc.sync.dma_start(t_wc[:], wt[ch - 1])
                        nxt = ch + 3
                        if nxt <= NCH and nxt not in wtiles:
                            _prefetch_w(nxt)
                        t_b = None
                        if do_exp:
                            t_b = bpool.tile([P, 2, 512], FP8, tag="bb", name="t_b")
                            nc.sync.dma_start(
                                t_b[:],
                                bdram[s][
                                    0 : 2 * P, ci * 512 : (ci + 1) * 512
                                ].rearrange("(c p) o -> p c o", p=P),
                            )
                        pair.append((ch, s, ci, t_wc, t_b))
                    sa, sb = pair[0][1], pair[1][1]
                    straddle = sa != sb
                    for mtl in range(MTL):
                        t_out = opool.tile([P, 2, 512], BF16, tag="o", name="t_out")
                        t_bg = None
                        if do_exp:
                            t_bg = gpool.tile([P, 2, 512], BF16, tag="g", name="t_bg")
                            idx_off = bass.IndirectOffsetOnAxis(
                                ap=t_idx[:, mtl : mtl + 1], axis=0
                            )
                            if straddle:
                                for j, (_, s, ci, _, _) in enumerate(pair):
                                    nc.gpsimd.indirect_dma_start(
                                        out=t_bg[:, j, :],
                                        out_offset=None,
                                        in_=cdram[s][:],
                                        in_offset=idx_off,
                                        element_offset=ci * 512,
                                    )
                            else:
                                nc.gpsimd.indirect_dma_start(
                                    out=t_bg[:].rearrange("p a b -> p (a b)"),
                                    out_offset=None,
                                    in_=cdram[sa][:],
                                    in_offset=idx_off,
                                    element_offset=pair[0][2] * 512,
                                )
                        # kk-interleaved across the pair: both chunks' matmuls
                        # share one x^T stationary per kk (deduped Ldweights)
                        p_ab = [
                            ps_b.tile([P, 512], F32, tag="b", name=f"p_{j}")
                            for j in range(2)
                        ]
                        for kk in range(KT):
                            for j in range(2):
                                nc.tensor.matmul(
                                    p_ab[j][:],
                                    t_xr[:, mtl, kk, :],
                                    pair[j][3][:, kk, :],
                                    start=(kk == 0),
                                    stop=(not do_exp and kk == KT - 1),
                                    skip_group_check=True,
                                )
                        if do_exp:
                            # one fp8 DoubleRow matmul contracts the full 256
                            # s-columns of the slice; paired chunks with the
                            # same slice share the stationary (deduped).
                            for j in range(2):
                                s = pair[j][1]
                                nc.tensor.matmul(
                                    p_ab[j][:],
                                    t_st[
                                        :, mtl, 2 * s * P : (2 * s + 2) * P
                                    ].rearrange("p (h t) -> p h t", h=2),
                                    pair[j][4][:],
                                    start=False,
                                    stop=True,
                                    perf_mode=DR,
                                    skip_group_check=True,
                                )
                        for j, (ch, s, ci, t_wc, t_b) in enumerate(pair):
                            if do_exp:
                                nc.vector.tensor_tensor(
                                    t_out[:, j, :],
                                    p_ab[j][:],
                                    t_bg[:, j, :],
                                    op=mybir.AluOpType.add,
                                )
                            elif (ch + mtl) % 2 == 0:
                                nc.vector.tensor_copy(t_out[:, j, :], p_ab[j][:])
                            else:
                                nc.scalar.copy(t_out[:, j, :], p_ab[j][:])
                        nc.scalar.dma_start(
                            out[
                                mtl * P : (mtl + 1) * P,
                                2 * pr * 512 : (2 * pr + 2) * 512,
                            ],
                            t_out[:].rearrange("p a b -> p (a b)"),
                        )

    import os

    if os.environ.get("KERNEL_NO_DEDUPE"):
        print("dedupe disabled via KERNEL_NO_DEDUPE")
    else:
        n_del = _dedupe_ldweights(nc)
        print(f"deduped {n_del} InstLdweights")
    nc.compile()
    return nc


# ---------------------------------------------------------------------------
# host-side sharding / unsharding
# ---------------------------------------------------------------------------


def _bf16(a):
    import ml_dtypes

    return np.asarray(a, np.float32).astype(ml_dtypes.bfloat16)


def _fp8(a):
    import ml_dtypes

    return np.asarray(a, np.float32).astype(ml_dtypes.float8_e4m3)


def shard_inputs(x, W, lora_a1, lora_a2, lora_b1, lora_b2, bias1, bias2, indices):
    indices = np.asarray(indices, np.int32)

    # base weight chunks: wt[ch, p, kk, j] = W[ch*512 + j, kk*128 + p]
    Wf = np.asarray(W, np.float32)
    wt = _bf16(
        np.ascontiguousarray(Wf.T.reshape(KT, P, NCH, 512).transpose(2, 1, 0, 3))
    )

    # fp8 shrink weights (A1|A2 flattened, pre-scaled by 1/ASCALE)
    a1f = np.asarray(lora_a1, np.float32).reshape(L * R, D)
    a2f = np.asarray(lora_a2, np.float32).reshape(L * R, D)
    aaug = np.concatenate([a1f, a2f], axis=0) * ASCALE  # [512, D]
    w8 = _fp8(np.ascontiguousarray(aaug.T.reshape(KT, P, 512).transpose(1, 0, 2)))

    def bmat(lb):
        bf = np.asarray(lb, np.float32).transpose(0, 2, 1).reshape(L * R, O)
        return _fp8(bf * ASCALE)

    b1m = bmat(lora_b1)
    b2m = bmat(lora_b2)

    # xt[c][mtl, p, kk, m] = x[c*1024 + mtl*128 + m, kk*128 + p]
    xtt = np.asarray(x, np.float32).reshape(NCORES, MTL, P, KT, P).transpose(
        0, 1, 4, 3, 2
    )
    xts = _bf16(xtt)
    x8s = _fp8(xtt)
    idxs = indices.reshape(NCORES, MTL, P).transpose(0, 2, 1)

    c1m = _bf16(bias1)
    c2m = _bf16(bias2)

    in_maps = []
    for c in range(NCORES):
        in_maps.append(
            {
                "xt": np.ascontiguousarray(xts[c]),
                "x8": np.ascontiguousarray(x8s[c]),
                "wt": wt,
                "w8": w8,
                "b1": b1m,
                "b2": b2m,
                "c1": c1m,
                "c2": c2m,
                "idx": np.ascontiguousarray(idxs[c]),
            }
        )
    return in_maps


def unshard_output(results):
    out = np.empty((T, NF), np.float32)
    for c in range(NCORES):
        out[c * TL : (c + 1) * TL, :] = np.asarray(results[c]["out"], np.float32)
    return out


_CACHE = {}


def get_nc():
    if "nc" not in _CACHE:
        _CACHE["nc"] = build_nc()
    return _CACHE["nc"]


def kernel(**inputs):
    from concourse import bass2jax

    nc = get_nc()
    in_maps = shard_inputs(**inputs)
    results = bass2jax.run_bass_via_pjrt(nc, in_maps, n_cores=NCORES)
    return unshard_output(results)
